# revision 12
# baseline (speedup 1.0000x reference)
"""GCN (2-layer MLP encoder + 2 GCNConv + classifier) on 8 TRN2 NeuronCores.

Strategy:
  - Nodes dealt round-robin by degree rank across 8 cores (12500 each,
    padded to 12544 = 98 tiles x 128). Position ordering is degree-sorted,
    so 128-node ELL tiles have tight max-degree.
  - Gather table rows are (core, partition-major) ordered; the table is the
    concatenation of per-core layer outputs, written contiguously.
  - Edge aggregation: per-dst-node ELL slots, gathered with dma_gather
    (int16 indices) in 4 windows of 25088 rows (= core pairs). A host-side
    greedy balances each node's in-edges across windows (cap ceil(deg/4))
    to minimize ELL padding.
  - Per tile: strided DVE reduces per window -> combine -> scale by
    dinv -> PE transpose -> augmented matmul (bias row) -> Lrelu on ACT
    (optionally scaled by dinv to produce the next layer's table rows).
  - 3 launches: MLP encoder; GCN layer 1; GCN layer 2 + classifier.
    The two GCN launches share one compiled NEFF (weights/scales are data).
"""

import numpy as np

N = 100000
E = 1200000
D_IN = 512
H = 128
HID = 64
C_CLS = 2
SLOPE = 0.01

NCORES = 8
NPC = 12500          # real nodes per core
NPOS = 12544         # padded positions per core (98 tiles x 128)
NT = 98              # tiles per core
P = 128
WINROWS = 2 * NPOS   # 25088 rows per window (= one core pair)
NWIN = 4
PAD_LOCAL = 84 * NT + 97   # 8329: a dummy node's row (local to any window)
MAX_GROUP_CHUNKS = 160     # slab budget per gather group (x256B/partition)
MAX_GROUP_TILES = 16

_cache = {}


# ----------------------------------------------------------------- host prep

def _greedy_windows(deg, src, dst, order, sweeps=2):
    """Window assignment. Node order[8g+j] (j=0..7): exactly 2 nodes of each
    rank-group g go to each window; cores 2w/2w+1 within the window. Greedy
    init minimizes sum cnt^2 over (dst, window) cells, then pair-swap
    refinement sweeps."""
    cnt = np.zeros((N, NWIN), np.int32)
    win_of = np.full(N, -1, np.int32)

    s_order = np.argsort(src, kind="stable")
    d_sorted = dst[s_order]
    indptr = np.searchsorted(src[s_order], np.arange(N + 1))

    BATCH = 2000
    for b0 in range(0, N, BATCH):
        nodes = order[b0:b0 + BATCH]
        lens = indptr[nodes + 1] - indptr[nodes]
        flat_d = (np.concatenate(
            [d_sorted[indptr[n]:indptr[n + 1]] for n in nodes])
            if lens.sum() else np.zeros(0, np.int64))
        seg = np.repeat(np.arange(len(nodes)), lens)
        scores = np.zeros((len(nodes), NWIN), np.int64)
        if len(flat_d):
            np.add.at(scores, seg, 2 * cnt[flat_d] + 1)
        offs = np.concatenate([[0], np.cumsum(lens)])
        for gb in range(0, len(nodes), 8):
            gn = min(8, len(nodes) - gb)
            taken = np.zeros(NWIN, np.int32)
            js = sorted(range(gn),
                        key=lambda j: -(offs[gb + j + 1] - offs[gb + j]))
            for j in js:
                row = scores[gb + j]
                bw, bv = -1, None
                for w in range(NWIN):
                    if taken[w] >= 2:
                        continue
                    if bv is None or row[w] < bv:
                        bw, bv = w, row[w]
                taken[bw] += 1
                n = nodes[gb + j]
                win_of[n] = bw
                dd = d_sorted[indptr[n]:indptr[n + 1]]
                if len(dd):
                    np.add.at(cnt, (dd, bw), 1)

    for _ in range(sweeps):
        for b0 in range(0, N, BATCH):
            nodes = order[b0:b0 + BATCH]
            for gb in range(0, len(nodes), 8):
                grp = nodes[gb:gb + min(8, len(nodes) - gb)]
                for a in range(len(grp)):
                    for b in range(a + 1, len(grp)):
                        na, nb = grp[a], grp[b]
                        wa, wb = win_of[na], win_of[nb]
                        if wa == wb:
                            continue
                        da = d_sorted[indptr[na]:indptr[na + 1]]
                        db = d_sorted[indptr[nb]:indptr[nb + 1]]
                        delta = (2 * cnt[da, wb] + 1
                                 - (2 * cnt[da, wa] - 1)).sum() \
                            + (2 * cnt[db, wa] + 1
                               - (2 * cnt[db, wb] - 1)).sum()
                        if delta < 0:
                            np.add.at(cnt, (da, wa), -1)
                            np.add.at(cnt, (da, wb), 1)
                            np.add.at(cnt, (db, wb), -1)
                            np.add.at(cnt, (db, wa), 1)
                            win_of[na], win_of[nb] = wb, wa

    # core within window: rank-group slot order
    core_of = np.full(N, -1, np.int32)
    for b0 in range(0, N, 8):
        grp = order[b0:b0 + 8]
        taken = np.zeros(NWIN, np.int32)
        for n in grp:
            w = win_of[n]
            core_of[n] = 2 * w + taken[w]
            taken[w] += 1
    return core_of, cnt


def preprocess(x, edge_index):
    rng = np.random.default_rng(1234)
    src = np.asarray(edge_index[0], np.int64)
    dst = np.asarray(edge_index[1], np.int64)
    deg = np.bincount(dst, minlength=N).astype(np.int64)
    dinv = (1.0 / np.sqrt(deg + 1.0)).astype(np.float32)

    order = np.argsort(-deg, kind="stable")       # rank -> node
    core_of, cnt = _greedy_windows(deg, src, dst, order)
    # position within core: pattern-sorted so ELL tiles are homogeneous
    pos_of = np.empty(N, np.int64)
    for c in range(NCORES):
        nodes_c = np.where(core_of == c)[0]
        v = cnt[nodes_c]
        key = np.lexsort((v[:, 3], v[:, 2], v[:, 1], v[:, 0],
                          v.max(1), v.sum(1)))[::-1]
        pos_of[nodes_c[key]] = np.arange(len(nodes_c))
    # table row (local within core): partition-major
    t_of = pos_of // P
    p_of = pos_of % P
    lrow_of = p_of * NT + t_of                     # 0..12543
    grow_of = core_of * NPOS + lrow_of             # global table row
    win_of = core_of // 2

    # --- ELL build ---------------------------------------------------------
    ce = core_of[dst]
    te = t_of[dst].astype(np.int32)
    pe = p_of[dst].astype(np.int32)
    we = win_of[src].astype(np.int32)
    val = (grow_of[src] - we.astype(np.int64) * WINROWS).astype(np.int32)
    assert val.min() >= 0 and val.max() < WINROWS

    # slot index k per (core, tile, window, partition): cumcount via lexsort
    key_order = np.lexsort((pe, we, te, ce))
    ce_s, te_s, we_s, pe_s, val_s = (a[key_order] for a in (ce, te, we, pe, val))
    grp = (((ce_s * NT + te_s) * NWIN + we_s) * P + pe_s)
    # cumcount within equal grp (grp is sorted)
    first = np.concatenate([[True], grp[1:] != grp[:-1]])
    idx_in_grp = np.arange(len(grp)) - np.maximum.accumulate(
        np.where(first, np.arange(len(grp)), 0))
    counts = np.bincount(grp, minlength=NCORES * NT * NWIN * P) \
        .reshape(NCORES, NT, NWIN, P)
    K = counts.max(axis=(0, 3))                    # [NT, NWIN] static slot counts

    # fill idx grids: for (c,t,w): [K_tw, P] int16, pad -> PAD_LOCAL
    ell = {}
    for t in range(NT):
        for w in range(NWIN):
            if K[t, w]:
                ell[(t, w)] = np.full((NCORES, K[t, w], P), PAD_LOCAL, np.int32)
    # vectorized scatter into ell: bucket edges by (t, w) once
    tw_s = te_s * NWIN + we_s
    tw_order = np.argsort(tw_s, kind="stable")
    tw_sorted = tw_s[tw_order]
    tw_ptr = np.searchsorted(tw_sorted, np.arange(NT * NWIN + 1))
    kk = idx_in_grp
    for t in range(NT):
        for w in range(NWIN):
            if (t, w) not in ell:
                continue
            sl = tw_order[tw_ptr[t * NWIN + w]:tw_ptr[t * NWIN + w + 1]]
            ell[(t, w)][ce_s[sl], kk[sl], pe_s[sl]] = val_s[sl]

    # --- group structure: pack tiles under a chunk budget -------------------
    groups = []
    cur, cur_chunks = [], 0
    for t in range(NT):
        kt = int(K[t].sum())
        if cur and (cur_chunks + kt > MAX_GROUP_CHUNKS
                    or len(cur) >= MAX_GROUP_TILES):
            groups.append(cur)
            cur, cur_chunks = [], 0
        cur.append(t)
        cur_chunks += kt
    if cur:
        groups.append(cur)
    # per (group, window): chunk offsets per tile and total
    gw_counts = [[int(sum(K[t, w] for t in tl)) for w in range(NWIN)]
                 for tl in groups]

    # per-core per-window flattened idx arrays (concatenated over groups)
    win_tot = [int(sum(gw_counts[g][w] for g in range(len(groups))))
               for w in range(NWIN)]
    idx_data = []
    for c in range(NCORES):
        per_win = []
        for w in range(NWIN):
            parts = []
            for tl in groups:
                for t in tl:
                    if K[t, w]:
                        parts.append(ell[(t, w)][c])     # [K_tw, P]
            flat = (np.concatenate(parts, axis=0).reshape(-1)
                    if parts else np.zeros(0, np.int32))  # slot-major
            assert len(flat) == win_tot[w] * P
            wrapped = flat.reshape(-1, 16).T if len(flat) else np.zeros((16, 0), np.int32)
            per_win.append(np.tile(wrapped, (8, 1)).astype(np.int16))
        idx_data.append(per_win)

    # --- per-core node data -------------------------------------------------
    nodes_of_core = [np.where(core_of == c)[0] for c in range(NCORES)]
    xT = []
    dinv_pm = []
    for c in range(NCORES):
        nodes = nodes_of_core[c]
        posv = pos_of[nodes]
        xc = np.zeros((NPOS, D_IN), np.float32)
        xc[posv] = x[nodes]                        # position-ordered
        xT.append(np.ascontiguousarray(xc.T))      # [512, 12544]
        dv = np.zeros((P, NT), np.float32)
        dv[p_of[nodes], t_of[nodes]] = dinv[nodes]
        dinv_pm.append(dv)

    return dict(
        core_of=core_of, pos_of=pos_of, p_of=p_of, t_of=t_of,
        K=K, groups=groups, gw_counts=gw_counts, win_tot=win_tot,
        idx_data=idx_data, xT=xT, dinv_pm=dinv_pm,
        nodes_of_core=nodes_of_core,
    )


# ------------------------------------------------------------- bass kernels

def _bass_mods():
    import concourse.bacc as bacc
    import concourse.tile as tile
    import concourse.mybir as mybir
    from concourse.bass_utils import run_bass_kernel_spmd
    from concourse.masks import make_identity
    from concourse import library_config
    return bacc, tile, mybir, run_bass_kernel_spmd, make_identity, library_config


def build_mlp(reps=1):
    bacc, tile, mybir, _, _, _ = _bass_mods()
    f32 = mybir.dt.float32
    nc = bacc.Bacc("TRN2", target_bir_lowering=False, debug=False,
                   enable_asserts=True, num_devices=NCORES)
    xT = nc.dram_tensor("xT", (D_IN, NPOS), f32, kind="ExternalInput").ap()
    w1 = nc.dram_tensor("w1", (D_IN, H), f32, kind="ExternalInput").ap()
    w2 = nc.dram_tensor("w2", (H, HID), f32, kind="ExternalInput").ap()
    b1 = nc.dram_tensor("b1", (H, 1), f32, kind="ExternalInput").ap()
    b2b = nc.dram_tensor("b2b", (P, HID), f32, kind="ExternalInput").ap()
    dv = nc.dram_tensor("dv", (P, NT), f32, kind="ExternalInput").ap()
    out = nc.dram_tensor("out", (NPOS, HID), f32, kind="ExternalOutput").ap()

    with tile.TileContext(nc) as tc:
        with tc.tile_pool(name="pers", bufs=1) as pers, \
             tc.tile_pool(name="sbuf", bufs=3) as pool, \
             tc.tile_pool(name="psum", bufs=2, space="PSUM") as psum:
            w1t = pers.tile([P, 4 * H], f32)
            nc.sync.dma_start(
                out=w1t[:].rearrange("p (c j) -> p c j", c=4, j=H),
                in_=w1.rearrange("(c p) j -> p c j", c=4, p=P))
            w2t = pers.tile([H, HID], f32)
            nc.sync.dma_start(out=w2t[:], in_=w2)
            b1t = pers.tile([H, 1], f32)
            nc.sync.dma_start(out=b1t[:], in_=b1)
            b2t = pers.tile([P, HID], f32)
            nc.sync.dma_start(out=b2t[:], in_=b2b)
            dvt = pers.tile([P, NT], f32)
            nc.sync.dma_start(out=dvt[:], in_=dv)
            obuf = pers.tile([P, NT * HID], f32)

            def body(_iv=None):
                mlp_body(nc, tc, mybir, pool, psum,
                         xT, w1t, w2t, b1t, b2t, dvt, obuf)
            if reps == 1:
                body()
            else:
                with tc.For_i(0, reps, 1):
                    body()
            nc.sync.dma_start(
                out=out.rearrange("(p t) j -> p t j", p=P, t=NT),
                in_=obuf[:].rearrange("p (t j) -> p t j", t=NT, j=HID))
    nc.compile()
    return nc


def mlp_body(nc, tc, mybir, pool, psum, xT, w1t, w2t, b1t, b2t, dvt, obuf):
    f32 = mybir.dt.float32
    for t in range(NT):
                xt = pool.tile([P, 4 * P], f32, tag="xt", name=f"xt{t}")
                nc.sync.dma_start(
                    out=xt[:].rearrange("p (c i) -> p c i", c=4, i=P),
                    in_=xT.rearrange("(c p) n -> p c n", c=4, p=P)[
                        :, :, t * P:(t + 1) * P])
                h1ps = psum.tile([P, H], f32, tag="h1ps", name=f"h1ps{t}",
                                 space="PSUM")
                for c4 in range(4):
                    nc.tensor.matmul(
                        out=h1ps[:], lhsT=w1t[:, c4 * H:(c4 + 1) * H],
                        rhs=xt[:, c4 * P:(c4 + 1) * P],
                        start=(c4 == 0), stop=(c4 == 3))
                h1t = pool.tile([H, P], f32, tag="h1t", name=f"h1t{t}")
                nc.scalar.activation(out=h1t[:], in_=h1ps[:],
                                     func=mybir.ActivationFunctionType.Lrelu,
                                     bias=b1t[:, :1], alpha=SLOPE)
                h2ps = psum.tile([P, HID], f32, tag="h2ps", name=f"h2ps{t}",
                                 space="PSUM")
                nc.tensor.matmul(out=h2ps[:], lhsT=h1t[:], rhs=w2t[:],
                                 start=True, stop=True)
                h2b = pool.tile([P, HID], f32, tag="h2b", name=f"h2b{t}")
                nc.vector.tensor_tensor(out=h2b[:], in0=h2ps[:], in1=b2t[:],
                                        op=mybir.AluOpType.add)
                nc.scalar.activation(out=obuf[:, t * HID:(t + 1) * HID],
                                     in_=h2b[:],
                                     func=mybir.ActivationFunctionType.Lrelu,
                                     scale=dvt[:, t:t + 1], alpha=SLOPE)


def build_gcn(K, groups, gw_counts, win_tot, reps=1, skip=frozenset()):
    bacc, tile, mybir, _, make_identity, library_config = _bass_mods()
    f32 = mybir.dt.float32
    i16 = mybir.dt.int16
    nc = bacc.Bacc("TRN2", target_bir_lowering=False, debug=False,
                   enable_asserts=True, num_devices=NCORES)
    tbl = nc.dram_tensor("tbl", (NCORES * NPOS, HID), f32,
                         kind="ExternalInput").ap()
    hloc = nc.dram_tensor("hloc", (NPOS, HID), f32, kind="ExternalInput").ap()
    wga = nc.dram_tensor("wga", (HID + 1, HID), f32, kind="ExternalInput").ap()
    wca = nc.dram_tensor("wca", (HID + 1, C_CLS), f32, kind="ExternalInput").ap()
    dv = nc.dram_tensor("dv", (P, NT), f32, kind="ExternalInput").ap()
    scl = nc.dram_tensor("scl", (P, NT), f32, kind="ExternalInput").ap()
    idx_in = [nc.dram_tensor(f"idx{w}", (P, max(1, win_tot[w] * 8)), i16,
                             kind="ExternalInput").ap() for w in range(NWIN)]
    out = nc.dram_tensor("out", (NPOS, HID), f32, kind="ExternalOutput").ap()
    logit = nc.dram_tensor("logit", (NPOS, C_CLS), f32, kind="ExternalOutput").ap()

    max_cg = max(sum(gwc) for gwc in gw_counts)

    with tile.TileContext(nc) as tc:
        with tc.tile_pool(name="pers", bufs=1) as pers, \
             tc.tile_pool(name="slabs", bufs=2) as slabp, \
             tc.tile_pool(name="work", bufs=3) as work, \
             tc.tile_pool(name="psum", bufs=2, space="PSUM") as psum:
            nc.gpsimd.load_library(library_config.mlp)
            wgat = pers.tile([HID + 1, HID], f32)
            nc.sync.dma_start(out=wgat[:], in_=wga)
            wcat = pers.tile([HID + 1, C_CLS], f32)
            nc.sync.dma_start(out=wcat[:], in_=wca)
            dvt = pers.tile([P, NT], f32)
            nc.sync.dma_start(out=dvt[:], in_=dv)
            sclt = pers.tile([P, NT], f32)
            nc.sync.dma_start(out=sclt[:], in_=scl)
            ident = pers.tile([P, P], f32)
            make_identity(nc, ident[:])
            hl = pers.tile([P, NT * HID], f32)
            nc.sync.dma_start(
                out=hl[:].rearrange("p (t j) -> p t j", t=NT, j=HID),
                in_=hloc.rearrange("(p t) j -> p t j", p=P, t=NT))
            idxt = []
            for w in range(NWIN):
                it = pers.tile([P, max(1, win_tot[w] * 8)], i16,
                               name=f"idxt{w}")
                nc.sync.dma_start(out=it[:], in_=idx_in[w])
                idxt.append(it)
            obuf = pers.tile([P, NT * HID], f32)
            lbuf = pers.tile([P, NT * C_CLS], f32)
            aug = [pers.tile([HID + 1, P], f32, name=f"aug{i}")
                   for i in range(4)]
            for a in aug:
                nc.vector.memset(a[HID:HID + 1, :], 1.0)

            def gcn_body(_iv=None):
                gcn_layer_body(nc, tc, mybir, K, groups, gw_counts, max_cg,
                               slabp, work, psum, tbl, idxt, hl, dvt, sclt,
                               wgat, wcat, ident, aug, obuf, lbuf, skip)
            if reps == 1:
                gcn_body()
            else:
                with tc.For_i(0, reps, 1):
                    gcn_body()

            nc.sync.dma_start(
                out=out.rearrange("(p t) j -> p t j", p=P, t=NT),
                in_=obuf[:].rearrange("p (t j) -> p t j", t=NT, j=HID))
            nc.sync.dma_start(
                out=logit.rearrange("(p t) j -> p t j", p=P, t=NT),
                in_=lbuf[:].rearrange("p (t j) -> p t j", t=NT, j=C_CLS))
    nc.compile()
    return nc


def gcn_layer_body(nc, tc, mybir, K, groups, gw_counts, max_cg,
                   slabp, work, psum, tbl, idxt, hl, dvt, sclt,
                   wgat, wcat, ident, aug, obuf, lbuf, skip=frozenset()):
    f32 = mybir.dt.float32
    if True:
            woff = [0] * NWIN   # running chunk offset per window (for idx slices)
            for gi, tl in enumerate(groups):
                cg = sum(gw_counts[gi])
                slab = slabp.tile([P, max_cg * HID], f32, tag="slab",
                                  name=f"slab{gi}")
                # gather calls, one per window
                off_w = []
                o = 0
                for w in range(NWIN):
                    off_w.append(o)
                    cgw = gw_counts[gi][w]
                    if cgw == 0 or "gather" in skip:
                        woff[w] += cgw
                        o += cgw
                        continue
                    ni = cgw * P
                    nc.gpsimd.dma_gather(
                        slab[:].rearrange("p (c j) -> p c j",
                                          c=max_cg, j=HID)[:, o:o + cgw, :],
                        tbl[w * WINROWS:(w + 1) * WINROWS, :],
                        idxt[w][:, woff[w] * 8:(woff[w] + cgw) * 8],
                        ni, ni, HID, single_packet=False,
                    )
                    woff[w] += cgw
                    o += cgw
                # per-tile compute
                toff = [0] * NWIN
                for t in tl:
                    if "tiles" in skip:
                        break
                    p5 = work.tile([P, 5 * HID], f32, tag="p5", name=f"p5_{t}")
                    nc.scalar.activation(
                        out=p5[:, 0:HID], in_=hl[:, t * HID:(t + 1) * HID],
                        func=mybir.ActivationFunctionType.Copy)
                    for w in range(NWIN):
                        ktw = int(K[t, w])
                        dstsl = p5[:, (1 + w) * HID:(2 + w) * HID]
                        if ktw == 0:
                            nc.vector.memset(dstsl, 0.0)
                            continue
                        start = off_w[w] + toff[w]
                        v = slab[:].rearrange(
                            "p (c j) -> p c j", c=max_cg, j=HID)[
                            :, start:start + ktw, :].rearrange(
                            "p c j -> p j c")
                        if "reduce" not in skip:
                            nc.vector.tensor_reduce(out=dstsl, in_=v,
                                                    axis=mybir.AxisListType.X,
                                                    op=mybir.AluOpType.add)
                        toff[w] += ktw
                    t1 = work.tile([P, HID], f32, tag="t1", name=f"t1_{t}")
                    nc.vector.tensor_reduce(
                        out=t1[:],
                        in_=p5[:].rearrange("p (c j) -> p j c", c=5, j=HID),
                        axis=mybir.AxisListType.X, op=mybir.AluOpType.add)
                    t2 = work.tile([P, HID], f32, tag="t2", name=f"t2_{t}")
                    nc.vector.tensor_scalar_mul(t2[:], t1[:], dvt[:, t:t + 1])
                    tps = psum.tile([HID, P], f32, tag="tps", name=f"tps{t}",
                                    space="PSUM")
                    nc.tensor.transpose(out=tps[:], in_=t2[:], identity=ident[:])
                    a = aug[t % 2]
                    nc.scalar.activation(out=a[0:HID, :], in_=tps[:],
                                         func=mybir.ActivationFunctionType.Copy)
                    qps = psum.tile([P, HID], f32, tag="qps", name=f"qps{t}",
                                    space="PSUM")
                    nc.tensor.matmul(out=qps[:], lhsT=a[:], rhs=wgat[:],
                                     start=True, stop=True)
                    nc.scalar.activation(out=obuf[:, t * HID:(t + 1) * HID],
                                         in_=qps[:],
                                         func=mybir.ActivationFunctionType.Lrelu,
                                         scale=sclt[:, t:t + 1], alpha=SLOPE)
                    if "head" in skip:
                        continue
                    # classifier head (used on layer 2 only; cheap)
                    gps = psum.tile([HID, P], f32, tag="gps", name=f"gps{t}",
                                    space="PSUM")
                    nc.tensor.transpose(out=gps[:],
                                        in_=obuf[:, t * HID:(t + 1) * HID],
                                        identity=ident[:])
                    a2 = aug[2 + t % 2]
                    nc.scalar.activation(out=a2[0:HID, :], in_=gps[:],
                                         func=mybir.ActivationFunctionType.Copy)
                    lps = psum.tile([P, C_CLS], f32, tag="lps", name=f"lps{t}",
                                    space="PSUM")
                    nc.tensor.matmul(out=lps[:], lhsT=a2[:], rhs=wcat[:],
                                     start=True, stop=True)
                    nc.scalar.activation(
                        out=lbuf[:, t * C_CLS:(t + 1) * C_CLS], in_=lps[:],
                        func=mybir.ActivationFunctionType.Copy)


# ------------------------------------------------------------------- driver

def kernel(x, W1, b1, W2, b2, Wg1, bg1, Wg2, bg2, Wc, bc, edge_index):
    _, _, _, run_bass_kernel_spmd, _, _ = _bass_mods()
    x = np.asarray(x, np.float32)

    pre = preprocess(x, edge_index)
    K, groups, gw_counts, win_tot = (pre["K"], pre["groups"],
                                     pre["gw_counts"], pre["win_tot"])

    if "mlp" not in _cache:
        _cache["mlp"] = build_mlp()
    gkey = ("gcn", K.tobytes())
    if gkey not in _cache:
        _cache[gkey] = build_gcn(K, groups, gw_counts, win_tot)
    nc_mlp, nc_gcn = _cache["mlp"], _cache[gkey]

    W1 = np.asarray(W1, np.float32)
    W2 = np.asarray(W2, np.float32)
    b2b = np.tile(np.asarray(b2, np.float32)[None, :], (P, 1))
    wg1a = np.concatenate([np.asarray(Wg1, np.float32),
                           np.asarray(bg1, np.float32)[None, :]], axis=0)
    wg2a = np.concatenate([np.asarray(Wg2, np.float32),
                           np.asarray(bg2, np.float32)[None, :]], axis=0)
    wca = np.concatenate([np.asarray(Wc, np.float32),
                          np.asarray(bc, np.float32)[None, :]], axis=0)
    ones_scl = np.ones((P, NT), np.float32)

    cores = list(range(NCORES))

    # ---- launch 1: MLP
    in_maps = [dict(xT=pre["xT"][c], w1=W1,
                    w2=W2, b1=np.asarray(b1, np.float32)[:, None],
                    b2b=b2b, dv=pre["dinv_pm"][c]) for c in cores]
    res = run_bass_kernel_spmd(nc_mlp, in_maps, core_ids=cores)
    table1 = np.concatenate([res.results[c]["out"] for c in cores], axis=0)

    def gcn_maps(table, wg, sclv):
        return [dict(tbl=table, hloc=table[c * NPOS:(c + 1) * NPOS],
                     wga=wg, wca=wca, dv=pre["dinv_pm"][c], scl=sclv[c],
                     **{f"idx{w}": (pre["idx_data"][c][w]
                                    if win_tot[w] else
                                    np.zeros((P, 1), np.int16))
                        for w in range(NWIN)})
                for c in cores]

    # ---- launch 2: GCN layer 1 (output scaled by dinv -> next table)
    res = run_bass_kernel_spmd(
        nc_gcn, gcn_maps(table1, wg1a, pre["dinv_pm"]), core_ids=cores)
    table2 = np.concatenate([res.results[c]["out"] for c in cores], axis=0)

    # ---- launch 3: GCN layer 2 + classifier (unscaled output)
    res = run_bass_kernel_spmd(
        nc_gcn, gcn_maps(table2, wg2a, [ones_scl] * NCORES), core_ids=cores)

    h_full = np.empty((N, HID), np.float32)
    logits_full = np.empty((N, C_CLS), np.float32)
    for c in cores:
        nodes = pre["nodes_of_core"][c]
        lrow = pre["p_of"][nodes] * NT + pre["t_of"][nodes]
        h_full[nodes] = res.results[c]["out"][lrow]
        logits_full[nodes] = res.results[c]["logit"][lrow]
    return logits_full, h_full
